# revision 5
# baseline (speedup 1.0000x reference)
"""Trainium2 kernel for nn_Decoder: seq2seq decoder w/ additive attention.

Strategy:
- Teacher forcing prob = 1.0 means the big vocab projection ([B*T,1280]@[1280,8000],
  ~168 GFLOP, 262 MB output) does NOT feed the recurrence -> batched outside the scan.
- The sequential scan (attention + RNN, ~1.5 GFLOP total) runs on host in fp32 numpy.
- The vocab projection + ELU runs on 8 NeuronCores, data-parallel over batch
  (8 batches/core), bias folded in as an extra contraction row, bf16 x bf16 -> fp32 psum.
"""
import sys
import numpy as np

sys.path.insert(0, "/opt/trn_rl_repo")

B, S, T = 64, 128, 128
V, EMB, ENC, DEC, HE = 8000, 256, 512, 512, 20
XD = EMB + DEC + ENC          # 1280
NCORES = 8
BLOC = B // NCORES            # 8 batches per core
MTOK = BLOC * T               # 1024 tokens per core
KCH = 11                      # 10 real k-chunks of 128 + 1 bias/ones chunk
NT, NCOL = 16, 500            # vocab tiled as 16 x 500

_CACHE = {}


def _elu(x):
    return np.where(x > 0, x, np.expm1(np.minimum(x, 0.0)))


def _recurrence(enc_hid, real_output, W_e1, b_e1, W_e2, b_e2, emb, W_rnn, b_rnn):
    """fp32 numpy replica of the scan; returns X [T,B,1280] and weights [B,T,S]."""
    encpart = (enc_hid @ W_e1[:ENC]) + b_e1          # [B,S,HE]
    We1b = W_e1[ENC:]                                 # [DEC,HE]
    h = np.zeros((B, DEC), np.float32)
    last = np.zeros((B,), np.int64)
    X = np.empty((T, B, XD), np.float32)
    Wts = np.empty((B, T, S), np.float32)
    for t in range(T):
        hid_part = h @ We1b                           # [B,HE]
        e = _elu(encpart + hid_part[:, None, :])      # [B,S,HE]
        f = _elu((e @ W_e2)[..., 0] + b_e2[0])        # [B,S]
        ew = np.exp(f)
        w = ew / ew.sum(axis=1, keepdims=True)        # [B,S]
        ctx = (w[:, None, :] @ enc_hid)[:, 0, :]      # [B,ENC]
        ch = emb[last % V]                            # [B,EMB]
        h_new = _elu(np.concatenate([ch, h, ctx], axis=1) @ W_rnn + b_rnn)
        X[t, :, :EMB] = ch
        X[t, :, EMB:EMB + DEC] = h_new
        X[t, :, EMB + DEC:] = ctx
        Wts[:, t, :] = w
        last = np.asarray(real_output[:, t]).astype(np.int64)
        h = h_new
    return X, Wts


def _build_nc():
    import concourse.bacc as bacc
    import concourse.mybir as mybir
    from concourse.tile import TileContext

    dt = mybir.dt
    nc = bacc.Bacc(
        "TRN2", target_bir_lowering=False, debug=False, num_devices=NCORES
    )
    xt = nc.declare_dram_parameter("xt", [KCH, 128, MTOK], dt.bfloat16, isOutput=False)
    wo = nc.declare_dram_parameter("wo", [KCH, 128, V], dt.bfloat16, isOutput=False)
    out = nc.declare_dram_parameter("out", [MTOK, V], dt.float32, isOutput=True)

    with TileContext(nc) as tc:
        with (
            tc.tile_pool(name="xpool", bufs=1) as xpool,
            tc.tile_pool(name="wpool", bufs=2) as wpool,
            tc.tile_pool(name="ppool", bufs=8, space="PSUM") as ppool,
            tc.tile_pool(name="epool", bufs=4) as epool,
        ):
            xtiles = []
            for k in range(KCH):
                xtile = xpool.tile([128, MTOK], dt.bfloat16, tag=f"x{k}")
                nc.sync.dma_start(out=xtile[:], in_=xt[k])
                xtiles.append(xtile)
            for n in range(NT):
                wts = []
                for k in range(KCH):
                    wt = wpool.tile([128, NCOL], dt.bfloat16, tag=f"w{k}")
                    nc.sync.dma_start(
                        out=wt[:], in_=wo[k][:, n * NCOL:(n + 1) * NCOL]
                    )
                    wts.append(wt)
                for m in range(MTOK // 128):
                    ps = ppool.tile([128, NCOL], dt.float32, tag="ps")
                    for k in range(KCH):
                        nc.tensor.matmul(
                            ps[:],
                            lhsT=xtiles[k][:, m * 128:(m + 1) * 128],
                            rhs=wts[k][:],
                            start=(k == 0),
                            stop=(k == KCH - 1),
                        )
                    # elu(z) = max(z,0) - 1 + exp(min(z,0))
                    tneg = epool.tile([128, NCOL], dt.float32, tag="tneg")
                    tpos = epool.tile([128, NCOL], dt.float32, tag="tpos")
                    texp = epool.tile([128, NCOL], dt.float32, tag="texp")
                    res = epool.tile([128, NCOL], dt.float32, tag="res")
                    nc.vector.tensor_scalar(
                        tneg[:], ps[:], 0.0, None, mybir.AluOpType.min
                    )
                    nc.vector.tensor_scalar(
                        tpos[:], ps[:], 0.0, -1.0,
                        mybir.AluOpType.max, mybir.AluOpType.add,
                    )
                    nc.scalar.activation(
                        texp[:], tneg[:], mybir.ActivationFunctionType.Exp
                    )
                    nc.vector.tensor_add(res[:], texp[:], tpos[:])
                    nc.sync.dma_start(
                        out=out[m * 128:(m + 1) * 128, n * NCOL:(n + 1) * NCOL],
                        in_=res[:],
                    )
    nc.finalize()
    return nc


def kernel(**inputs):
    from ml_dtypes import bfloat16
    from concourse.bass_utils import run_bass_kernel_spmd

    enc_hid = np.asarray(inputs["enc_hid"], np.float32)
    real_output = np.asarray(inputs["real_output"])
    W_e1 = np.asarray(inputs["W_e1"], np.float32)
    b_e1 = np.asarray(inputs["b_e1"], np.float32)
    W_e2 = np.asarray(inputs["W_e2"], np.float32)
    b_e2 = np.asarray(inputs["b_e2"], np.float32)
    emb = np.asarray(inputs["emb"], np.float32)
    W_rnn = np.asarray(inputs["W_rnn"], np.float32)
    b_rnn = np.asarray(inputs["b_rnn"], np.float32)
    W_out = np.asarray(inputs["W_out"], np.float32)
    b_out = np.asarray(inputs["b_out"], np.float32)

    X, Wts = _recurrence(
        enc_hid, real_output, W_e1, b_e1, W_e2, b_e2, emb, W_rnn, b_rnn
    )

    # Per-core stationary operand: X^T padded with a ones-row (bias trick).
    Xc = X.transpose(1, 0, 2)  # [B, T, XD]
    in_maps = []
    wpad = np.zeros((KCH * 128, V), np.float32)
    wpad[:XD] = W_out
    wpad[XD] = b_out
    wo_np = np.ascontiguousarray(wpad.reshape(KCH, 128, V).astype(bfloat16))
    for c in range(NCORES):
        xl = Xc[c * BLOC:(c + 1) * BLOC].reshape(MTOK, XD)
        xtp = np.zeros((KCH * 128, MTOK), np.float32)
        xtp[:XD] = xl.T
        xtp[XD] = 1.0
        in_maps.append({
            "xt": np.ascontiguousarray(xtp.reshape(KCH, 128, MTOK).astype(bfloat16)),
            "wo": wo_np,
        })

    if "nc" not in _CACHE:
        _CACHE["nc"] = _build_nc()
    res = run_bass_kernel_spmd(_CACHE["nc"], in_maps, list(range(NCORES))).results
    outs = np.concatenate(
        [np.asarray(r["out"], np.float32).reshape(BLOC, T, V) for r in res], axis=0
    )
    return outs, Wts
